# revision 1
# baseline (speedup 1.0000x reference)
"""Cross-attention kernel for Trainium2, SPMD over 8 NeuronCores.

Problem (hardcoded): B=32, N=2560 queries, Dq=512, Dc=1024, 8 heads x 64 dim,
context = 77 text + 16 image tokens, two attentions (text keys via W_k/W_v,
image keys via W_k_ip/W_v_ip) summed, then W_out projection + bias.

Sharding: data-parallel over batch, 4 batches per core, no collectives.
x and context are transposed host-side during sharding so every DMA lands in
the feature-on-partitions layout the matmuls need.

Per-core kernel (all matmul operands float32r: full-rate 1 cycle/row on PE,
~1e-4 matmul rel err; fp32 is 4x slower):
  phase 0: k^T = scale * (W_k|W_k_ip)^T @ ctx^T   [128 inner, 93 keys] tiles
           V   = ctx @ (W_v|W_v_ip)               [93 keys, 512 inner]
  per 512-query chunk (software-pipelined, stage X of chunk i overlaps
  stages of chunks i-1/i+1 so no engine FIFO ever blocks on a chain):
    P: q^T = W_q^T @ x^T                          (PE, 4 K-tiles)
    A: s^T[93 keys, 512 q] = k^T_h^T @ q^T_h; E = exp(s^T) (ScalarE)
    B: r[2, 512] = ind^T @ E  (txt/img key sums); r_inv = 1/r (VectorE)
    C: Bcast[93, 512] = ind2^T @ r_inv  (K=2 outer product re-broadcasts the
       normalizer across key partitions); P = E * Bcast (VectorE)
    D: O^T = V^T @ P  (single matmul over all 93 keys sums the text and
       image attention outputs); evacuate to attn^T (ScalarE)
    F: out = attn^T^T @ W_out + b_bcast (VectorE add) -> DMA out

PSUM budget (8 banks): big(q/final)=2, scores=2, sums=2, bcast=2.
"""

import sys

if "/opt/trn_rl_repo" not in sys.path:
    sys.path.insert(0, "/opt/trn_rl_repo")

import numpy as np

from concourse import bacc
import concourse.mybir as mybir
from concourse.tile import TileContext
from concourse.bass_utils import run_bass_kernel_spmd

F32 = mybir.dt.float32
F32R = mybir.dt.float32r
EXP = mybir.ActivationFunctionType.Exp

P = 128
NCORES = 8
B = 32
BPC = B // NCORES  # batches per core
N = 2560
DQ = 512
DC = 1024
H = 8
D = 64
INNER = H * D  # 512
TT = 77  # text tokens
TI = 16  # image tokens
T = TT + TI  # 93
CH = 512  # query chunk
NCH = N // CH  # 5
SCALE = D ** (-0.5)

_CACHED = None


def _build(cfg=None):
    cfg = cfg or {}
    xs_bufs = cfg.get("xs", 3)
    big_bufs = cfg.get("big", 2)
    pss_bufs = cfg.get("pss", 2)
    psr_bufs = cfg.get("psr", 2)
    psb_bufs = cfg.get("psb", 2)
    osb_bufs = cfg.get("osb", 6)
    import contextlib
    nc = bacc.Bacc("TRN2", target_bir_lowering=False, debug=False, num_devices=NCORES)

    xt_d = nc.dram_tensor("xT", [BPC, DQ, N], F32R, kind="ExternalInput").ap()
    ctxt_d = nc.dram_tensor("ctxT", [BPC, DC, T], F32R, kind="ExternalInput").ap()
    wq_d = nc.dram_tensor("W_q", [DQ, INNER], F32R, kind="ExternalInput").ap()
    wk_d = nc.dram_tensor("W_k", [DC, INNER], F32R, kind="ExternalInput").ap()
    wv_d = nc.dram_tensor("W_v", [DC, INNER], F32R, kind="ExternalInput").ap()
    wkip_d = nc.dram_tensor("W_k_ip", [DC, INNER], F32R, kind="ExternalInput").ap()
    wvip_d = nc.dram_tensor("W_v_ip", [DC, INNER], F32R, kind="ExternalInput").ap()
    wout_d = nc.dram_tensor("W_out", [INNER, DQ], F32R, kind="ExternalInput").ap()
    ind_d = nc.dram_tensor("ind", [T, 32], F32R, kind="ExternalInput").ap()
    ind2_d = nc.dram_tensor("ind2", [2, T], F32R, kind="ExternalInput").ap()
    bb_d = nc.dram_tensor("b_bcast", [P, DQ], F32, kind="ExternalInput").ap()
    out_d = nc.dram_tensor("out", [BPC, N, DQ], F32, kind="ExternalOutput").ap()

    with TileContext(nc) as tc:
        with (
            tc.tile_pool(name="persist", bufs=1) as pp,
            tc.tile_pool(name="ps_big", bufs=big_bufs, space="PSUM") as ps_big,
            tc.tile_pool(name="ps_ss", bufs=pss_bufs, space="PSUM") as ps_ss,
            tc.tile_pool(name="ps_r", bufs=psr_bufs, space="PSUM") as ps_r,
            tc.tile_pool(name="ps_b", bufs=psb_bufs, space="PSUM") as ps_b,
        ):
            ind_t = pp.tile([T, 32], F32R, tag="ind")
            ind2_t = pp.tile([2, T], F32R, tag="ind2")
            bb_t = pp.tile([P, DQ], F32, tag="bb")

            wq_all = pp.tile([P, 4, INNER], F32R, tag="wq_all")
            wout_all = pp.tile([P, 4, DQ], F32R, tag="wout_all")

            # K^T[b][m] : [128 inner-dims, 93 keys] (text keys 0:77 from W_k,
            # image keys 77:93 from W_k_ip), pre-scaled by 1/sqrt(d).
            # V[b] : [93 keys, 512 inner] (text rows via W_v, image via W_v_ip)
            kT = [
                [
                    pp.tile([P, T], F32R, tag=f"kT{b}_{m}", name=f"kT{b}_{m}")
                    for m in range(4)
                ]
                for b in range(BPC)
            ]
            V = [pp.tile([T, INNER], F32R, tag=f"v{b}", name=f"v{b}") for b in range(BPC)]

            # ---- pools for the main loop (opened early so chunk (0,0)
            # projection work can interleave with phase 0) ----
            wstack = contextlib.ExitStack()
            wp = wstack.enter_context(tc.tile_pool(name="work", bufs=2))
            xsp = wstack.enter_context(tc.tile_pool(name="xsp", bufs=xs_bufs))
            osp = wstack.enter_context(tc.tile_pool(name="osp", bufs=osb_bufs))
            ep = wstack.enter_context(tc.tile_pool(name="ework", bufs=16))
            rp = wstack.enter_context(tc.tile_pool(name="rwork", bufs=8))

            def emit_p(b, c):
                # x^T for this chunk straight from DRAM:
                # [128 partitions, kt, 512 tokens]
                xT = xsp.tile([P, 4, CH], F32R, tag="xT", name=f"xT{b}_{c}")
                nc.sync.dma_start(
                    xT[:],
                    xt_d[b].rearrange("(k p) t -> p k t", p=P)[
                        :, :, c * CH : (c + 1) * CH
                    ],
                )
                # q^T chunk: [128, m, 512]
                qT = wp.tile([P, 4, CH], F32R, tag="qT", name=f"qT{b}_{c}")
                for m in range(4):
                    psq = ps_big.tile([P, CH], F32, tag="big", name=f"psq{b}_{c}_{m}")
                    for kt in range(4):
                        nc.tensor.matmul(
                            psq[:],
                            lhsT=wq_all[:, kt, m * P : (m + 1) * P],
                            rhs=xT[:, kt, :],
                            start=(kt == 0),
                            stop=(kt == 3),
                        )
                    nc.scalar.copy(qT[:, m, :], psq[:])
                return (b, c, qT)

            # ---- phase 0: context projections ----
            with (
                tc.tile_pool(name="ph0", bufs=1) as p0,
                tc.tile_pool(name="ph0w", bufs=2) as p0w,
            ):
                # weight tiles share two rotating slots (tag w8); text-key
                # projection starts as soon as W_k and the contexts land
                wk_all = p0w.tile([P, 8, INNER], F32R, tag="w8", name="wk_all")
                nc.sync.dma_start(wk_all[:], wk_d.rearrange("(k p) n -> p k n", p=P))
                ctxT = []
                for b in range(BPC):
                    ct = p0.tile([P, 8, T], F32R, tag=f"ctxT{b}", name=f"ctxT{b}")
                    nc.sync.dma_start(
                        ct[:], ctxt_d[b].rearrange("(k p) t -> p k t", p=P)
                    )
                    ctxT.append(ct)
                nc.sync.dma_start(wq_all[:], wq_d.rearrange("(k p) n -> p k n", p=P))
                wv_all = p0w.tile([P, 8, INNER], F32R, tag="w8", name="wv_all")
                nc.sync.dma_start(wv_all[:], wv_d.rearrange("(k p) n -> p k n", p=P))

                # text keys: kT[:, :TT] (fp32r needs an even moving free dim:
                # project 78 keys, junk col 77 unused)
                for b in range(BPC):
                    for m in range(4):
                        pst = ps_ss.tile([P, CH], F32, tag="pss")
                        for kt in range(8):
                            nc.tensor.matmul(
                                pst[:, : TT + 1],
                                lhsT=wk_all[:, kt, m * P : (m + 1) * P],
                                rhs=ctxT[b][:, kt, : TT + 1],
                                start=(kt == 0),
                                stop=(kt == 7),
                            )
                        nc.scalar.mul(kT[b][m][:, :TT], pst[:, :TT], SCALE)

                wkip_all = p0w.tile([P, 8, INNER], F32R, tag="w8", name="wkip_all")
                nc.sync.dma_start(
                    wkip_all[:], wkip_d.rearrange("(k p) n -> p k n", p=P)
                )

                # text values: V[:TT, :]
                for b in range(BPC):
                    psv = ps_ss.tile([P, CH], F32, tag="pss")
                    for kt in range(8):
                        nc.tensor.matmul(
                            psv[:TT, :],
                            lhsT=ctxT[b][:, kt, :TT],
                            rhs=wv_all[:, kt, :],
                            start=(kt == 0),
                            stop=(kt == 7),
                        )
                    nc.scalar.copy(V[b][:TT, :], psv[:TT, :])

                pre_p = emit_p(0, 0)
                wvip_all = p0w.tile([P, 8, INNER], F32R, tag="w8", name="wvip_all")
                nc.sync.dma_start(
                    wvip_all[:], wvip_d.rearrange("(k p) n -> p k n", p=P)
                )
                nc.sync.dma_start(
                    wout_all[:], wout_d.rearrange("(k p) n -> p k n", p=P)
                )
                nc.sync.dma_start(ind_t[:], ind_d)
                nc.sync.dma_start(ind2_t[:], ind2_d)
                nc.sync.dma_start(bb_t[:], bb_d)

                # image keys: kT[:, TT:]
                for b in range(BPC):
                    for m in range(4):
                        psi = ps_big.tile([P, CH], F32, tag="big")
                        for kt in range(8):
                            nc.tensor.matmul(
                                psi[:, :TI],
                                lhsT=wkip_all[:, kt, m * P : (m + 1) * P],
                                rhs=ctxT[b][:, kt, TT:T],
                                start=(kt == 0),
                                stop=(kt == 7),
                            )
                        nc.scalar.mul(kT[b][m][:, TT:T], psi[:, :TI], SCALE)

                # image values: V[TT:, :] (engines cannot address partition
                # offset 77; bounce through SBUF + DMA)
                for b in range(BPC):
                    psw = ps_big.tile([P, CH], F32, tag="big")
                    for kt in range(8):
                        nc.tensor.matmul(
                            psw[:TI, :],
                            lhsT=ctxT[b][:, kt, TT:T],
                            rhs=wvip_all[:, kt, :],
                            start=(kt == 0),
                            stop=(kt == 7),
                        )
                    vtmp = p0.tile([TI, INNER], F32R, tag="vtmp", name=f"vtmp{b}")
                    nc.scalar.copy(vtmp[:], psw[:TI, :])
                    nc.sync.dma_start(V[b][TT:T, :], vtmp[:])

            # ---- main loop ----
            # Stage A (scores+exp) of each chunk is emitted one step AHEAD of
            # stages B/C/D/final of the previous chunk: while VectorE chews a
            # chunk's reciprocals/normalizations, the PE stream always has the
            # next chunk's independent projection/scores work in its queue.
            if True:

                def emit_a(pstate):
                    b, c, qT = pstate
                    # stage A: scores + exp for all heads
                    esbs = []
                    for h in range(H):
                        mt, mo = h // 2, 64 * (h % 2)
                        pss = ps_ss.tile([P, CH], F32, tag="pss")
                        nc.tensor.matmul(
                            pss[:T, :],
                            lhsT=kT[b][mt][mo : mo + 64, :],
                            rhs=qT[mo : mo + 64, mt, :],
                            start=True,
                            stop=True,
                            tile_position=(mo, 0),
                        )
                        esb = ep.tile([T, CH], F32R, tag="esb")
                        nc.scalar.activation(esb[:], pss[:T, :], EXP)
                        esbs.append(esb)
                    return (b, c, esbs)

                def emit_b(state):
                    b, c, esbs = state
                    # stage B: key-sums + reciprocal per head
                    rinvs = []
                    for h in range(H):
                        psr = ps_r.tile([2, CH], F32, tag="psr")
                        nc.tensor.matmul(
                            psr[:],
                            lhsT=ind_t[:, :2],
                            rhs=esbs[h][:],
                            start=True,
                            stop=True,
                        )
                        rinv = rp.tile([2, CH], F32R, tag="rinv")
                        with nc.allow_low_precision(
                            reason="float32r output is bit-compatible with fp32"
                        ):
                            nc.vector.reciprocal(rinv[:], psr[:])
                        rinvs.append(rinv)
                    return (b, c, esbs, rinvs)

                def emit_cdf(state):
                    b, c, esbs, rinvs = state
                    # stage C: broadcast + normalize per head
                    for h in range(H):
                        psb = ps_b.tile([T, CH], F32, tag="psb")
                        nc.tensor.matmul(
                            psb[:],
                            lhsT=ind2_t[:2, :],
                            rhs=rinvs[h][:],
                            start=True,
                            stop=True,
                        )
                        nc.vector.tensor_mul(
                            out=esbs[h][:], in0=esbs[h][:], in1=psb[:]
                        )

                    # stage D: attention output per head + evacuation
                    aT = wp.tile([P, 4, CH], F32R, tag="aT")
                    for h in range(H):
                        mt, mo = h // 2, 64 * (h % 2)
                        pso = ps_ss.tile([P, CH], F32, tag="pss")
                        nc.tensor.matmul(
                            pso[:D, :],
                            lhsT=V[b][:, h * D : (h + 1) * D],
                            rhs=esbs[h][:],
                            start=True,
                            stop=True,
                        )
                        nc.scalar.copy(aT[mo : mo + D, mt, :], pso[:D, :])

                    # final projection for this chunk
                    for m in range(4):
                        psf = ps_big.tile([P, CH], F32, tag="big")
                        for kt in range(4):
                            nc.tensor.matmul(
                                psf[:],
                                lhsT=aT[:, kt, m * P : (m + 1) * P],
                                rhs=wout_all[:, kt, :],
                                start=(kt == 0),
                                stop=(kt == 3),
                            )
                        osb = osp.tile([P, DQ], F32, tag="osb")
                        nc.vector.tensor_add(out=osb[:], in0=psf[:], in1=bb_t[:])
                        nc.sync.dma_start(
                            out_d[b, c * CH + m * P : c * CH + (m + 1) * P, :],
                            osb[:],
                        )

                coords = [(b, c) for b in range(BPC) for c in range(NCH)]
                pstates = {coords[0]: pre_p}
                pend = None
                last = len(coords) - 1
                for i, (b, c) in enumerate(coords):
                    if (b, c) not in pstates:
                        pstates[(b, c)] = emit_p(b, c)
                    state = emit_a(pstates.pop((b, c)))
                    bstate = emit_b(pend) if pend is not None else None
                    if i == last:
                        # shorten the tail: the final chunk's sums/recip go
                        # out right behind its scores
                        lastb = emit_b(state)
                    if i + 1 < len(coords):
                        pstates[coords[i + 1]] = emit_p(*coords[i + 1])
                    if bstate is not None:
                        emit_cdf(bstate)
                    pend = state
                emit_cdf(lastb)
            wstack.close()

    nc.compile()
    return nc


def _get_nc(cfg=None):
    global _CACHED
    if _CACHED is None:
        _CACHED = _build(cfg)
    return _CACHED


def _aux_inputs(b_out):
    ind = np.zeros((T, 32), dtype=np.float32)
    ind[:TT, 0] = 1.0
    ind[TT:, 1] = 1.0
    ind2 = np.zeros((2, T), dtype=np.float32)
    ind2[0, :TT] = 1.0
    ind2[1, TT:] = 1.0
    bb = np.broadcast_to(np.asarray(b_out, np.float32), (P, DQ)).copy()
    return ind, ind2, bb


def run(inputs, trace=False):
    x = np.asarray(inputs["x"], dtype=np.float32)
    ctx = np.asarray(inputs["context"], dtype=np.float32)
    xT = np.ascontiguousarray(x.transpose(0, 2, 1))
    ctxT = np.ascontiguousarray(ctx.transpose(0, 2, 1))
    ws = {
        k: np.ascontiguousarray(np.asarray(inputs[k], dtype=np.float32))
        for k in ("W_q", "W_k", "W_v", "W_k_ip", "W_v_ip", "W_out")
    }
    ind, ind2, bb = _aux_inputs(inputs["b_out"])

    in_maps = []
    for c in range(NCORES):
        m = {
            "xT": xT[c * BPC : (c + 1) * BPC],
            "ctxT": ctxT[c * BPC : (c + 1) * BPC],
            "ind": ind,
            "ind2": ind2,
            "b_bcast": bb,
        }
        m.update(ws)
        in_maps.append(m)

    nc = _get_nc()
    res = run_bass_kernel_spmd(nc, in_maps, list(range(NCORES)), trace=trace)
    out = np.concatenate([res.results[c]["out"] for c in range(NCORES)], axis=0)
    return out.astype(np.float32, copy=False), res


def kernel(**inputs):
    out, _ = run(inputs)
    return out



# revision 5
# speedup vs baseline: 1.1699x; 1.1699x over previous
"""Cross-attention kernel for Trainium2, SPMD over 8 NeuronCores.

Problem (hardcoded): B=32, N=2560 queries, Dq=512, Dc=1024, 8 heads x 64 dim,
context = 77 text + 16 image tokens, two attentions (text keys via W_k/W_v,
image keys via W_k_ip/W_v_ip) summed, then W_out projection + bias.

Sharding: data-parallel over batch, 4 batches per core, no collectives.

Per-core kernel, per 512-query chunk (software-pipelined across chunks):
  P: q^T = W_q^T @ x^T via fp8e4m3 DoubleRow matmuls (3-term error-
     compensated split: x8@W8 + xlo8@W8 + x8@Wlo8, all operands at a x32
     weight scale, un-scaled at psum evacuation; ~1.5e-3 max rel err).
  A: s^T[93, 512] = k^T_h^T @ q^T_h per head; E = exp(s^T) in bf16 (ScalarE).
  B (flipped): r^T[128q, 2] = E-slice^T @ ind via tiny 2-column matmuls,
     all 8 heads x 4 query-blocks batched into ONE [128, 8, 4, 2] psum ->
     ONE reciprocal (DVE) instead of 8 wide ones.
  T: PE-transpose rinv^T back to [16, 512] (4 blocks), evacuate bf16.
  C: bcast[93, 512] = mask_h^T @ rinv (per-head constant mask selects the
     head's txt/img rows); E *= bcast (VectorE).
  D: O^T = V^T @ E per head, EVEN/ODD head pairs share one [128, 512] psum
     (base partitions 0/64) -> one ScalarE evacuation per pair.
  F: out = aT^T @ W_out + b_bcast (VectorE add) -> DMA out.

Phase 0 projects context for all 4 batches at once: the key projections
stream a [128, 8, 372] batched ctx^T (308 text cols at full fp32r rate; the
64 image cols via bf16 to dodge the <256-free-dim fp32r penalty), and the
image values stack 4x16 key rows into one matmul.
"""

import sys

if "/opt/trn_rl_repo" not in sys.path:
    sys.path.insert(0, "/opt/trn_rl_repo")

import numpy as np
import ml_dtypes

from concourse import bacc
import concourse.mybir as mybir
from concourse.tile import TileContext
from concourse.bass_utils import run_bass_kernel_spmd

F32 = mybir.dt.float32
F32R = mybir.dt.float32r
BF16 = mybir.dt.bfloat16
F8 = mybir.dt.float8e4
DRMODE = mybir.MatmulPerfMode.DoubleRow
EXP = mybir.ActivationFunctionType.Exp

P = 128
NCORES = 8
B = 32
BPC = B // NCORES  # batches per core
N = 2560
DQ = 512
DC = 1024
H = 8
D = 64
INNER = H * D  # 512
TT = 77  # text tokens
TI = 16  # image tokens
T = TT + TI  # 93
CH = 512  # query chunk
NCH = N // CH  # 5
SCALE = D ** (-0.5)
WS = 32.0  # fp8 weight scale

_CACHED = None


def _build(cfg=None):
    cfg = cfg or {}
    xs_bufs = cfg.get("xs", 3)
    big_bufs = cfg.get("big", 2)
    work_bufs = cfg.get("work", 2)
    psb_bufs = cfg.get("psb", 2)
    osb_bufs = cfg.get("osb", 6)
    import contextlib

    nc = bacc.Bacc("TRN2", target_bir_lowering=False, debug=False, num_devices=NCORES)

    x8_d = nc.dram_tensor("x8", [BPC, P, 2, 2, N], F8, kind="ExternalInput").ap()
    x8lo_d = nc.dram_tensor("x8lo", [BPC, P, 2, 2, N], F8, kind="ExternalInput").ap()
    # batched ctx^T for all 4 batches: [p, kt, 308 txt | 64 img]
    ctx_d = nc.dram_tensor("ctxR", [P, 8, 372], F32R, kind="ExternalInput").ap()
    wq8_d = nc.dram_tensor("wq8", [P, 2, 2, INNER], F8, kind="ExternalInput").ap()
    wq8lo_d = nc.dram_tensor("wq8lo", [P, 2, 2, INNER], F8, kind="ExternalInput").ap()
    wk_d = nc.dram_tensor("W_k", [DC, INNER], F32R, kind="ExternalInput").ap()
    wv_d = nc.dram_tensor("W_v", [DC, INNER], F32R, kind="ExternalInput").ap()
    wkip8_d = nc.dram_tensor("W_k_ip_bf", [DC, INNER], BF16, kind="ExternalInput").ap()
    wvip_d = nc.dram_tensor("W_v_ip", [DC, INNER], F32R, kind="ExternalInput").ap()
    wout_d = nc.dram_tensor("W_out", [INNER, DQ], F32R, kind="ExternalInput").ap()
    ind93_d = nc.dram_tensor("ind93", [T, 2], BF16, kind="ExternalInput").ap()
    mask2_d = nc.dram_tensor("mask2", [16, H, T], BF16, kind="ExternalInput").ap()
    ident_d = nc.dram_tensor("ident", [P, P], F32R, kind="ExternalInput").ap()
    bb_d = nc.dram_tensor("b_bcast", [P, DQ], F32, kind="ExternalInput").ap()
    out_d = nc.dram_tensor("out", [BPC, N, DQ], F32, kind="ExternalOutput").ap()

    with TileContext(nc) as tc:
        with (
            tc.tile_pool(name="persist", bufs=1) as pp,
            tc.tile_pool(name="ps_big", bufs=big_bufs, space="PSUM") as ps_big,
            tc.tile_pool(name="ps_work", bufs=work_bufs, space="PSUM") as ps_work,
            tc.tile_pool(name="ps_b", bufs=psb_bufs, space="PSUM") as ps_b,
            tc.tile_pool(name="ps_rt", bufs=1, space="PSUM") as ps_rt,
            tc.tile_pool(name="ps_ri", bufs=1, space="PSUM") as ps_ri,
        ):
            ind93_t = pp.tile([T, 2], BF16, tag="ind93")
            mask2_t = pp.tile([16, H, T], BF16, tag="mask2")
            ident_t = pp.tile([P, P], F32R, tag="ident")
            bb_t = pp.tile([P, DQ], F32, tag="bb")

            wq8_t = pp.tile([P, 2, 2, INNER], F8, tag="wq8")
            wq8lo_t = pp.tile([P, 2, 2, INNER], F8, tag="wq8lo")
            wout_all = pp.tile([P, 4, DQ], F32R, tag="wout_all")

            # kT[p, m, b, t]: [128 inner-slice, 4 m, 4 batch, 93 keys],
            # pre-scaled by 1/sqrt(d). V[b]: [93 keys, 512 inner] bf16.
            kT = pp.tile([P, 4, BPC, T], F32R, tag="kT")
            V = [pp.tile([T, INNER], BF16, tag=f"v{b}", name=f"v{b}") for b in range(BPC)]

            wstack = contextlib.ExitStack()
            wp = wstack.enter_context(tc.tile_pool(name="work", bufs=2))
            xsp = wstack.enter_context(tc.tile_pool(name="xsp", bufs=xs_bufs))
            osp = wstack.enter_context(tc.tile_pool(name="osp", bufs=osb_bufs))
            ep = wstack.enter_context(tc.tile_pool(name="ework", bufs=16))
            rp = wstack.enter_context(tc.tile_pool(name="rwork", bufs=2))

            def emit_p(b, c):
                # x8 / x8lo chunks straight from DRAM: [128, kt, slot, 512]
                x8 = xsp.tile([P, 2, 2, CH], F8, tag="x8", name=f"x8_{b}_{c}")
                nc.sync.dma_start(x8[:], x8_d[b][:, :, :, c * CH : (c + 1) * CH])
                x8lo = xsp.tile([P, 2, 2, CH], F8, tag="x8lo", name=f"x8lo_{b}_{c}")
                nc.sync.dma_start(x8lo[:], x8lo_d[b][:, :, :, c * CH : (c + 1) * CH])
                # q^T chunk: [128, m, 512] = (x8@W8 + x8lo@W8 + x8@Wlo8)/32
                qT = wp.tile([P, 4, CH], F32R, tag="qT", name=f"qT{b}_{c}")
                for m in range(4):
                    psq = ps_big.tile([P, CH], F32, tag="big", name=f"psq{b}_{c}_{m}")
                    series = [(wq8_t, x8), (wq8_t, x8lo), (wq8lo_t, x8)]
                    for si, (wt, xt) in enumerate(series):
                        for kt in range(2):
                            nc.tensor.matmul(
                                psq[:],
                                lhsT=wt[:, kt, :, m * P : (m + 1) * P],
                                rhs=xt[:, kt, :, :],
                                start=(si == 0 and kt == 0),
                                stop=(si == 2 and kt == 1),
                                perf_mode=DRMODE,
                            )
                    nc.scalar.mul(qT[:, m, :], psq[:], 1.0 / WS)
                return (b, c, qT)

            # ---- phase 0: context projections (all 4 batches batched) ----
            with (
                tc.tile_pool(name="ph0", bufs=1) as p0,
                tc.tile_pool(name="ph0w", bufs=2) as p0w,
            ):
                ct = p0.tile([P, 8, 372], F32R, tag="ct", name="ct")
                nc.sync.dma_start(ct[:], ctx_d)
                wk_all = p0w.tile([P, 8, INNER], F32R, tag="w8", name="wk_all")
                nc.sync.dma_start(wk_all[:], wk_d.rearrange("(k p) n -> p k n", p=P))
                nc.sync.dma_start(wq8_t[:], wq8_d)
                nc.sync.dma_start(wq8lo_t[:], wq8lo_d)
                nc.sync.dma_start(ind93_t[:], ind93_d)
                nc.sync.dma_start(mask2_t[:], mask2_d)
                nc.sync.dma_start(ident_t[:], ident_d)
                nc.sync.dma_start(bb_t[:], bb_d)

                # text keys for all 4 batches: free dim 308 >= 256, full rate
                for m in range(4):
                    pst = ps_work.tile([P, CH], F32, tag="pss", name=f"pkt{m}")
                    for kt in range(8):
                        nc.tensor.matmul(
                            pst[:, : 4 * TT],
                            lhsT=wk_all[:, kt, m * P : (m + 1) * P],
                            rhs=ct[:, kt, : 4 * TT],
                            start=(kt == 0),
                            stop=(kt == 7),
                        )
                    nc.scalar.mul(
                        kT[:, m, :, :TT],
                        pst[:, : 4 * TT].rearrange("p (b t) -> p b t", b=4),
                        SCALE,
                    )

                # image-context bf16 copy + bf16 image-key weights
                ct8 = p0.tile([P, 8, 4 * TI], BF16, tag="ct8", name="ct8")
                nc.scalar.copy(ct8[:], ct[:, :, 4 * TT :])
                wkip8 = p0w.tile([P, 8, INNER], BF16, tag="w8b", name="wkip8")
                nc.sync.dma_start(wkip8[:], wkip8_d.rearrange("(k p) n -> p k n", p=P))
                wv_all = p0w.tile([P, 8, INNER], F32R, tag="w8", name="wv_all")
                nc.sync.dma_start(wv_all[:], wv_d.rearrange("(k p) n -> p k n", p=P))

                # image keys for all 4 batches (bf16: 64-col free at full rate)
                for m in range(4):
                    psi = ps_work.tile([P, CH], F32, tag="pss", name=f"pki{m}")
                    for kt in range(8):
                        nc.tensor.matmul(
                            psi[:, : 4 * TI],
                            lhsT=wkip8[:, kt, m * P : (m + 1) * P],
                            rhs=ct8[:, kt, :],
                            start=(kt == 0),
                            stop=(kt == 7),
                        )
                    nc.scalar.mul(
                        kT[:, m, :, TT:T],
                        psi[:, : 4 * TI].rearrange("p (b t) -> p b t", b=4),
                        SCALE,
                    )

                # text values per batch
                for b in range(BPC):
                    psv = ps_big.tile([P, CH], F32, tag="big", name=f"pv{b}")
                    for kt in range(8):
                        nc.tensor.matmul(
                            psv[:TT, :],
                            lhsT=ct[:, kt, b * TT : (b + 1) * TT],
                            rhs=wv_all[:, kt, :],
                            start=(kt == 0),
                            stop=(kt == 7),
                        )
                    nc.scalar.copy(V[b][:TT, :], psv[:TT, :])

                pre_p = emit_p(0, 0)

                wvip_all = p0w.tile([P, 8, INNER], F32R, tag="w8", name="wvip_all")
                nc.sync.dma_start(wvip_all[:], wvip_d.rearrange("(k p) n -> p k n", p=P))
                nc.sync.dma_start(wout_all[:], wout_d.rearrange("(k p) n -> p k n", p=P))

                # image values: all 4 batches stacked on partitions (4x16=64)
                psw = ps_big.tile([P, CH], F32, tag="big", name="pvi")
                for kt in range(8):
                    nc.tensor.matmul(
                        psw[: 4 * TI, :],
                        lhsT=ct[:, kt, 4 * TT :],
                        rhs=wvip_all[:, kt, :],
                        start=(kt == 0),
                        stop=(kt == 7),
                    )
                vtmp = p0.tile([4 * TI, INNER], BF16, tag="vtmp", name="vtmp")
                nc.scalar.copy(vtmp[:], psw[: 4 * TI, :])
                for b in range(BPC):
                    nc.sync.dma_start(V[b][TT:T, :], vtmp[b * TI : (b + 1) * TI, :])

            # ---- main loop ----
            def emit_a(pstate):
                b, c, qT = pstate
                # stage A: scores + exp (bf16) for all heads
                esbs = []
                for h in range(H):
                    mt, mo = h // 2, 64 * (h % 2)
                    pss = ps_work.tile([P, CH], F32, tag="pss")
                    nc.tensor.matmul(
                        pss[:T, :],
                        lhsT=kT[mo : mo + 64, mt, b, :],
                        rhs=qT[mo : mo + 64, mt, :],
                        start=True,
                        stop=True,
                        tile_position=(mo, 0),
                    )
                    esb = ep.tile([T, CH], BF16, tag="esb")
                    nc.scalar.activation(esb[:], pss[:T, :], EXP)
                    esbs.append(esb)
                return (b, c, esbs)

            def emit_b(state):
                b, c, esbs = state
                # stage B (flipped): per-(head, block) 2-col sums batched into
                # one [128, 8, 4, 2] psum, then ONE reciprocal + PE transpose
                rps = ps_rt.tile([P, 4, H, 2], F32, tag="rt")
                for h in range(H):
                    for blk in range(4):
                        nc.tensor.matmul(
                            rps[:, blk, h, :],
                            lhsT=esbs[h][:, blk * P : (blk + 1) * P],
                            rhs=ind93_t[:],
                            start=True,
                            stop=True,
                        )
                rinvT = rp.tile([P, 4, H, 2], F32R, tag="rinvT")
                with nc.allow_low_precision(
                    reason="float32r output is bit-compatible with fp32"
                ):
                    nc.vector.reciprocal(rinvT[:], rps[:])
                rips = ps_ri.tile([16, CH], F32R, tag="rinvps")
                for blk in range(4):
                    nc.tensor.transpose(
                        rips[:, blk * P : (blk + 1) * P],
                        rinvT[:, blk, :, :],
                        ident_t[:],
                    )
                rinv_sb = rp.tile([16, CH], BF16, tag="rinvsb")
                nc.scalar.copy(rinv_sb[:], rips[:])
                return (b, c, esbs, rinv_sb)

            def emit_cdf(state):
                b, c, esbs, rinv_sb = state
                # stage C: per-head masked broadcast + normalize
                for h in range(H):
                    psb = ps_b.tile([T, CH], F32, tag="psb")
                    nc.tensor.matmul(
                        psb[:],
                        lhsT=mask2_t[:, h, :],
                        rhs=rinv_sb[:],
                        start=True,
                        stop=True,
                    )
                    nc.vector.tensor_mul(out=esbs[h][:], in0=esbs[h][:], in1=psb[:])

                # stage D: attention output, head pairs share one psum
                aT = wp.tile([P, 4, CH], F32R, tag="aT")
                for m in range(4):
                    pso = ps_work.tile([P, CH], F32, tag="pss")
                    for odd in range(2):
                        h = 2 * m + odd
                        nc.tensor.matmul(
                            pso[64 * odd : 64 * odd + 64, :],
                            lhsT=V[b][:, h * D : (h + 1) * D],
                            rhs=esbs[h][:],
                            start=True,
                            stop=True,
                        )
                    nc.scalar.copy(aT[:, m, :], pso[:])

                # final projection for this chunk
                for m in range(4):
                    psf = ps_big.tile([P, CH], F32, tag="big")
                    for kt in range(4):
                        nc.tensor.matmul(
                            psf[:],
                            lhsT=aT[:, kt, m * P : (m + 1) * P],
                            rhs=wout_all[:, kt, :],
                            start=(kt == 0),
                            stop=(kt == 3),
                        )
                    osb = osp.tile([P, DQ], F32, tag="osb")
                    nc.vector.tensor_add(out=osb[:], in0=psf[:], in1=bb_t[:])
                    nc.sync.dma_start(
                        out_d[b, c * CH + m * P : c * CH + (m + 1) * P, :],
                        osb[:],
                    )

            coords = [(b, c) for b in range(BPC) for c in range(NCH)]
            pstates = {coords[0]: pre_p}
            pend = None
            last = len(coords) - 1
            for i, (b, c) in enumerate(coords):
                if (b, c) not in pstates:
                    pstates[(b, c)] = emit_p(b, c)
                state = emit_a(pstates.pop((b, c)))
                bstate = emit_b(pend) if pend is not None else None
                if i == last:
                    lastb = emit_b(state)
                if i + 1 < len(coords):
                    pstates[coords[i + 1]] = emit_p(*coords[i + 1])
                if bstate is not None:
                    emit_cdf(bstate)
                pend = state
            emit_cdf(lastb)
            wstack.close()

    nc.compile()
    return nc


def _get_nc(cfg=None):
    global _CACHED
    if _CACHED is None:
        _CACHED = _build(cfg)
    return _CACHED


F8NP = ml_dtypes.float8_e4m3
BF16NP = ml_dtypes.bfloat16


def _prep_inputs(inputs):
    x = np.asarray(inputs["x"], dtype=np.float32)
    ctx = np.asarray(inputs["context"], dtype=np.float32)
    ws = {
        k: np.ascontiguousarray(np.asarray(inputs[k], dtype=np.float32))
        for k in ("W_q", "W_k", "W_v", "W_k_ip", "W_v_ip", "W_out")
    }

    # x8 / x8lo in DoubleRow layout [b, p, kt, slot, n]; dim = 128*(2kt+slot)+p
    xT = x.transpose(0, 2, 1)  # [B, 512, N]
    xT_dr = xT.reshape(B, 2, 2, P, N).transpose(0, 3, 1, 2, 4)
    x8 = xT_dr.astype(F8NP)
    x8lo = (xT_dr - x8.astype(np.float32)).astype(F8NP)

    # wq8 / wq8lo: [p, kt, slot, col] at 32x scale
    wq32 = (WS * ws["W_q"]).reshape(2, 2, P, INNER).transpose(2, 0, 1, 3)
    wq8 = wq32.astype(F8NP)
    wq8lo = (wq32 - wq8.astype(np.float32)).astype(F8NP)

    # batched ctx^T: [p, kt, 308 txt | 64 img] per core (built per-core below)
    ctxT = np.ascontiguousarray(ctx.transpose(0, 2, 1))  # [B, 1024, 93]

    ind93 = np.zeros((T, 2), dtype=BF16NP)
    ind93[:TT, 0] = 1
    ind93[TT:, 1] = 1
    mask2 = np.zeros((16, H, T), dtype=BF16NP)
    for h in range(H):
        mask2[2 * h, h, :TT] = 1
        mask2[2 * h + 1, h, TT:] = 1
    ident = np.eye(P, dtype=np.float32)
    bb = np.broadcast_to(np.asarray(inputs["b_out"], np.float32), (P, DQ)).copy()

    core_maps = []
    for cidx in range(NCORES):
        bs = slice(cidx * BPC, (cidx + 1) * BPC)
        cb = ctxT[bs].reshape(BPC, 8, P, T).transpose(2, 1, 0, 3)  # [p, kt, b, t]
        ctx_r = np.concatenate(
            [
                cb[:, :, :, :TT].reshape(P, 8, BPC * TT),
                cb[:, :, :, TT:].reshape(P, 8, BPC * TI),
            ],
            axis=2,
        )
        m = {
            "x8": np.ascontiguousarray(x8[bs]),
            "x8lo": np.ascontiguousarray(x8lo[bs]),
            "ctxR": np.ascontiguousarray(ctx_r),
            "wq8": np.ascontiguousarray(wq8),
            "wq8lo": np.ascontiguousarray(wq8lo),
            "W_k": ws["W_k"],
            "W_v": ws["W_v"],
            "W_k_ip_bf": ws["W_k_ip"].astype(BF16NP),
            "W_v_ip": ws["W_v_ip"],
            "W_out": ws["W_out"],
            "ind93": ind93,
            "mask2": mask2,
            "ident": ident,
            "b_bcast": bb,
        }
        core_maps.append(m)
    return core_maps


def run(inputs, trace=False):
    in_maps = _prep_inputs(inputs)
    nc = _get_nc()
    res = run_bass_kernel_spmd(nc, in_maps, list(range(NCORES)), trace=trace)
    out = np.concatenate([res.results[c]["out"] for c in range(NCORES)], axis=0)
    return out.astype(np.float32, copy=False), res


def kernel(**inputs):
    out, _ = run(inputs)
    return out
